# revision 20
# baseline (speedup 1.0000x reference)
"""Single-head attention (B=8, S=2048, H=1024, D=64) on 8 TRN2 NeuronCores.

Sharding: data-parallel over the batch dim - core b computes batch element b.

v5: fp16 datapath + cross-rep software pipeline.
  Engine budget per rep: PE ~49us of matmul, ScalarE ~33us of exp, Pool
  ~24us of f32->f16 convert, DMA ~24us of x load. The binding constraint
  is PE, but only if the ScalarE-bound attention phase overlaps the
  PE-bound prep phase. v4 ran prep (transpose + qk proj) at the top of
  each rep with ScalarE idle, then attention with the PE half idle. v5
  interleaves rep i+1's prep into rep i's attention blocks:
    jb0: v-projection(i)        jb1: x load+convert(i+1)
    jb2: transpose(i+1) b0-1    jb3: transpose(i+1) b2-3 + qk(i+1) b0-3
  so in steady state every engine streams continuously.

  fp16 instead of bf16: same PE/DVE throughput (1 cyc/row, 2x DVE), but
  all tensor magnitudes here are < 500 so the 3 extra mantissa bits are
  free accuracy (rel err 4.4e-4 vs 3.5e-3).

  fp8_scores=True computes scores with e4m3 q/k in DoubleRow perf mode
  (2 contraction rows/cycle; rel err 1.31e-2, still under the 2e-2
  gate). Measured on HW it is timing-neutral -- the PE does not reach
  the modeled 2x on these 256-cycle matmuls -- so it stays off.

  The prologue (and the repeats=1 graded path) chains per s-block:
  load(b) -> convert(b) -> transpose(b) -> qk(b), so attention starts
  after ~1.5 blocks of load instead of after the full x load.

Dataflow per core (all PE operands f16, f32 PSUM accumulation):
  xT[h, s]  = PE-transpose of f16 x tiles
  q8/k8     = wqk.T @ xT (+bias on PSUM evac, direct to [64, slot, S])
  vT        = wv.T @ xT (+bias); v_aug = [v*mask | mask | 0pad] in [k, 128]
  attnT     = exp(scoresT/8)                        (f16, ScalarE)
  outT_aug  = sum_k v_aug.T @ attnT                 ([128, SB] f32 PSUM)
  out       = transpose(outT_aug) * 1/denominator, one f32 store/rep
"""

import sys

sys.path.insert(0, "/opt/trn_rl_repo")

import numpy as np

B, S, H, D = 8, 2048, 1024, 64
SB = 512          # s-block (streaming block of queries)
NBLK = S // SB    # 4
NT = S // 128     # 16 t-tiles (and s-tiles)
HC = H // 128     # 8 h-chunks


def build_nc(repeats=1, probe_no_load=False, load_engines=("sync",),
             fp8_scores=False, cast_load=False, probe_load_only=False,
             early_load=False, conv_split=False):
    import concourse.bacc as bacc
    import concourse.mybir as mybir
    import concourse.tile as tile
    from concourse.masks import make_identity

    dt = mybir.dt
    f32, bf16, i32 = dt.float32, dt.float16, dt.int32
    f8 = dt.float8e4
    DR = mybir.MatmulPerfMode.DoubleRow
    AF = mybir.ActivationFunctionType

    nc = bacc.Bacc("TRN2", target_bir_lowering=False, debug=False, num_devices=8)

    X = nc.dram_tensor("x_b", [S, H], f32, kind="ExternalInput")
    MASK = nc.dram_tensor("mask_b", [S], i32, kind="ExternalInput")
    WQ = nc.dram_tensor("Wq", [H, D], f32, kind="ExternalInput")
    BQ = nc.dram_tensor("bq", [D], f32, kind="ExternalInput")
    WK = nc.dram_tensor("Wk", [H, D], f32, kind="ExternalInput")
    BK = nc.dram_tensor("bk", [D], f32, kind="ExternalInput")
    WV = nc.dram_tensor("Wv", [H, D], f32, kind="ExternalInput")
    BV = nc.dram_tensor("bv", [D], f32, kind="ExternalInput")
    OUT = nc.dram_tensor("out_b", [S, D], f32, kind="ExternalOutput")

    with tile.TileContext(nc) as tc:
        with (
            tc.tile_pool(name="const", bufs=1) as cpool,
            tc.tile_pool(name="xs", bufs=2) as xs_pool,
            tc.tile_pool(name="x16", bufs=2) as x16_pool,
            tc.tile_pool(name="xt", bufs=1) as xt_pool,
            tc.tile_pool(name="stk", bufs=2) as stk_pool,
            tc.tile_pool(name="vps", bufs=2) as v_pool,
            tc.tile_pool(name="attn", bufs=2) as at_pool,
            tc.tile_pool(name="outs", bufs=2) as o_pool,
            tc.tile_pool(name="ps_tr", bufs=2, space="PSUM") as ps_tr,
            tc.tile_pool(name="ps_sc", bufs=2, space="PSUM") as ps_sc,
            tc.tile_pool(name="ps_acc", bufs=2, space="PSUM") as ps_acc,
        ):
            # ---- constants ----
            ident_f = cpool.tile([128, 128], f32)
            make_identity(nc, ident_f)
            ident = cpool.tile([128, 128], bf16)
            nc.vector.tensor_copy(ident, ident_f)

            # fused [Wq | Wk] stationary (bf16): one projection matmul makes q and k
            wstage = cpool.tile([128, HC, 2 * D], f32, tag="wstage")
            nc.gpsimd.dma_start(out=wstage[:, :, 0:D], in_=WQ.ap().rearrange("(c p) m -> p c m", p=128))
            nc.gpsimd.dma_start(out=wstage[:, :, D:2 * D], in_=WK.ap().rearrange("(c p) m -> p c m", p=128))
            wqk = cpool.tile([128, HC, 2 * D], bf16)
            nc.vector.tensor_copy(wqk, wstage)
            wstage2 = cpool.tile([128, HC, 2 * D], f32, tag="wstage")
            nc.gpsimd.dma_start(out=wstage2[:, :, 0:D], in_=WV.ap().rearrange("(c p) m -> p c m", p=128))
            wv = cpool.tile([128, HC, D], bf16)
            nc.vector.tensor_copy(wv, wstage2[:, :, 0:D])

            bias_qk = cpool.tile([128, 1], f32)
            bias_v = cpool.tile([D, 1], f32)
            nc.gpsimd.dma_start(out=bias_qk[0:D, :], in_=BQ.ap().rearrange("(p o) -> p o", o=1))
            nc.gpsimd.dma_start(out=bias_qk[D:2 * D, :], in_=BK.ap().rearrange("(p o) -> p o", o=1))
            nc.gpsimd.dma_start(out=bias_v, in_=BV.ap().rearrange("(p o) -> p o", o=1))

            # key mask as 1.0/0.0 per t-chunk column (folded into v_aug)
            mask_i = cpool.tile([128, NT], i32)
            nc.gpsimd.dma_start(out=mask_i, in_=MASK.ap().rearrange("(c p) -> p c", p=128))
            mask_f = cpool.tile([128, NT], f32)
            nc.vector.tensor_copy(mask_f, mask_i)
            mask_m = cpool.tile([128, NT], f32)
            nc.vector.tensor_scalar(
                out=mask_m, in0=mask_f,
                scalar1=0.0, scalar2=None,
                op0=mybir.AluOpType.not_equal,
            )

            # v_aug [k-part, t-tile, 128]: cols 0:64 = masked v (rewritten each
            # iteration), col 64 = mask (the softmax-denominator ones column),
            # cols 65:128 = zero pad (full 128 stationary cols enable the PE's
            # fast-weight-load path). Cols 64:128 are written ONCE here.
            v_aug = cpool.tile([128, NT, 128], bf16, tag="v_aug")
            nc.vector.memset(v_aug[:, :, D + 1:128], 0.0)
            for i in range(NT):
                nc.vector.tensor_copy(v_aug[:, i, D:D + 1], mask_m[:, i:i + 1])

            # ---- pipeline stage helpers ----
            def load_block(x16, jb):
                # 2 DMAs (+ Pool converts unless the DMA itself casts).
                # cast_load="split" puts one DMA on the gpsimd SWDGE queue
                # (casting) and one on sync HWDGE (+convert) so the two
                # queues drain the load concurrently.
                for u in range(2):
                    if cast_load is True or (cast_load == "split" and u == 0):
                        # gpsimd SWDGE casts f32->f16 in the DMA CCE: no
                        # staging tile, no Pool convert stage
                        nc.gpsimd.dma_start(
                            out=x16[:, jb * 4 + u * 2:jb * 4 + (u + 1) * 2, :],
                            in_=X.ap().rearrange(
                                "(b u t p) h -> p b u t h", p=128, b=NBLK, u=2
                            )[:, jb, u, :, :],
                        )
                        continue
                    eng = getattr(nc, load_engines[(jb * 2 + u) % len(load_engines)])
                    xs = xs_pool.tile([128, 2, H], f32, tag="xs")
                    eng.dma_start(
                        out=xs,
                        in_=X.ap().rearrange(
                            "(b u t p) h -> p b u t h", p=128, b=NBLK, u=2
                        )[:, jb, u, :, :],
                    )
                    ceng = nc.vector if (conv_split and u == 1) else nc.gpsimd
                    ceng.tensor_copy(
                        x16[:, jb * 4 + u * 2:jb * 4 + (u + 1) * 2, :], xs
                    )

            def transpose_block(xt, x16, jb):
                # PE-transpose the 4 s-tiles x 8 h-chunks of block jb
                for cg in range(HC // 2):
                    ps = ps_tr.tile([128, 2, 4, 128], bf16, tag="tr")
                    for h in range(2):
                        c = 2 * cg + h
                        for st in range(4):
                            nc.tensor.transpose(
                                ps[:, h, st, :],
                                x16[:, jb * 4 + st, c * 128:(c + 1) * 128],
                                ident,
                            )
                    nc.vector.tensor_copy(
                        xt[:, 2 * cg:2 * cg + 2, jb * 4:(jb + 1) * 4, :], ps
                    )

            # q8/k8: fp8-e4m3 copies of qT/kT in DoubleRow layout [64, 2, S].
            # Slot 0 carries data; k8 slot 1 is zeroed so the second DoubleRow
            # contraction subtile contributes nothing (q8 slot 1 is junk -- it
            # multiplies k8's zeros). The zero/junk slots are written only on
            # the first allocation of each physical buffer (bufs=2 cycling).
            qk8_init = {"q8": 0, "k8": 0}

            def qk8_tiles():
                sdt = f8 if fp8_scores else bf16
                q8 = stk_pool.tile([D, 2, S], sdt, tag="q8")
                k8 = stk_pool.tile([D, 2, S], sdt, tag="k8")
                if qk8_init["q8"] < 2:
                    qk8_init["q8"] += 1
                    qk8_init["k8"] += 1
                    nc.vector.memset(q8[:, 1, :], 0.0)
                    nc.vector.memset(k8[:, 1, :], 0.0)
                return q8, k8

            def qk_block(q8, k8, xt, j):
                # qk projection; bias-add evacuates PSUM straight to fp8
                # (or f16 when fp8_scores=False)
                sl = slice(j * SB, (j + 1) * SB)
                ps = ps_acc.tile([128, SB], f32, tag="acc")
                for c in range(HC):
                    nc.tensor.matmul(
                        ps, wqk[:, c, :], xt[:, c, j * 4:(j + 1) * 4, :],
                        start=(c == 0), stop=(c == HC - 1),
                    )
                nc.vector.tensor_scalar_add(q8[:, 0, sl], ps[0:D, :], bias_qk[0:D])
                nc.vector.tensor_scalar_add(k8[:, 0, sl], ps[D:2 * D, :], bias_qk[D:2 * D])

            def pass1(jb, q8, k8):
                # fp8 DoubleRow scores (2 contraction rows/cycle) + exp
                sl = slice(jb * SB, (jb + 1) * SB)
                at = at_pool.tile([128, NT // 2, 2, SB], bf16, tag="at")
                for ih in range(NT // 2):
                    ps = ps_sc.tile([128, 2, SB], f32, tag="sc")
                    i0, i1 = ih, ih + NT // 2
                    pm = DR if fp8_scores else None
                    nc.tensor.matmul(
                        ps[:, 0, :],
                        k8[:, 0:(2 if fp8_scores else 1), i0 * 128:(i0 + 1) * 128],
                        q8[:, 0:(2 if fp8_scores else 1), sl],
                        start=True, stop=True, perf_mode=pm,
                    )
                    nc.tensor.matmul(
                        ps[:, 1, :],
                        k8[:, 0:(2 if fp8_scores else 1), i1 * 128:(i1 + 1) * 128],
                        q8[:, 0:(2 if fp8_scores else 1), sl],
                        start=True, stop=True, perf_mode=pm,
                    )
                    nc.scalar.activation(
                        out=at[:, ih, :, :], in_=ps, func=AF.Exp, scale=0.125,
                    )
                return at

            def project_v(xt):
                for j in range(NBLK):
                    sl = slice(j * SB, (j + 1) * SB)
                    ps_v = ps_acc.tile([128, SB], f32, tag="acc")
                    for c in range(HC):
                        nc.tensor.matmul(
                            ps_v[0:D, :], wv[:, c, :], xt[:, c, j * 4:(j + 1) * 4, :],
                            start=(c == 0), stop=(c == HC - 1),
                        )
                    vT = v_pool.tile([D, S], bf16, tag="vT")
                    nc.vector.tensor_scalar_add(vT[:, sl], ps_v[0:D, :], bias_v)
                    pst = ps_tr.tile([128, 2, 4, 128], bf16, tag="tr")
                    for st in range(4):
                        i = j * 4 + st
                        nc.tensor.transpose(
                            pst[:, 0, st, 0:D], vT[:, i * 128:(i + 1) * 128], ident[0:D, 0:D]
                        )
                    for st in range(4):
                        i = j * 4 + st
                        nc.vector.tensor_scalar_mul(
                            v_aug[:, i, 0:D], pst[:, 0, st, 0:D], mask_m[:, i:i + 1]
                        )

            def pass2(jb, at, outbuf):
                # attn@v + transpose + normalize for block jb
                ps_o = ps_acc.tile([128, SB], f32, tag="acc")
                for i in range(NT):
                    nc.tensor.matmul(
                        ps_o, v_aug[:, i, :],
                        at[:, i % (NT // 2), i // (NT // 2), :],
                        start=(i == 0), stop=(i == NT - 1),
                    )
                o_t = o_pool.tile([96, SB], bf16, tag="ot")
                nc.vector.tensor_copy(o_t, ps_o[0:96, :])
                pst = ps_tr.tile([128, 4, 96], bf16, tag="tr")
                for st in range(4):
                    nc.tensor.transpose(
                        pst[:, st, :], o_t[:, st * 128:(st + 1) * 128], ident[0:96, 0:96]
                    )
                recip4 = o_pool.tile([128, 4, 1], f32, tag="recip")
                nc.vector.reciprocal(recip4, pst[:, :, D:D + 1])
                for st in range(4):
                    nc.vector.tensor_scalar_mul(
                        outbuf[:, jb * 4 + st, :], pst[:, st, 0:D], recip4[:, st, :]
                    )

            def make_xt_stack(x16):
                """Transpose + qk-project all blocks, chained per block."""
                xt = xt_pool.tile([128, HC, NT, 128], bf16, tag="xt")
                q8, k8 = qk8_tiles()
                for jb in range(NBLK):
                    transpose_block(xt, x16, jb)
                    qk_block(q8, k8, xt, jb)
                return xt, q8, k8

            if probe_load_only:
                # timing probe: just the x load stream, nothing else
                sink = cpool.tile([128, 1], f32, tag="sink")
                for rep in range(repeats):
                    x16p = x16_pool.tile([128, NT, H], bf16, tag="x16")
                    for jb in range(NBLK):
                        load_block(x16p, jb)
                    nc.vector.tensor_copy(sink, x16p[:, 0, 0:1])
                nc.sync.dma_start(
                    out=OUT.ap().rearrange("(t p) d -> p t d", p=128)[:, 0:1, 0:1],
                    in_=sink.rearrange("p (t o) -> p t o", t=1),
                )
                nc.compile()
                return nc

            # ---- software pipeline across repeats ----
            # prologue: rep 0's load + prep, per-block chained
            x16 = x16_pool.tile([128, NT, H], bf16, tag="x16")
            for jb in range(NBLK):
                load_block(x16, jb)
            xt, q8, k8 = make_xt_stack(x16)

            x16_next = None
            for rep in range(repeats):
                fetch_next = rep + 1 < repeats and not probe_no_load
                outbuf = o_pool.tile([128, NT, D], f32, tag="outbuf")
                prev = None
                for jb in range(NBLK):
                    at = pass1(jb, q8, k8)
                    if jb == 0:
                        project_v(xt)
                        if fetch_next and x16_next is None:
                            # start i+1's x load under the attention phase
                            x16_next = x16_pool.tile([128, NT, H], bf16, tag="x16")
                            load_block(x16_next, 0)
                            load_block(x16_next, 1)
                    if jb == 1 and fetch_next:
                        load_block(x16_next, 2)
                        load_block(x16_next, 3)
                    if jb == 2 and fetch_next:
                        # i+1 prep starts: xt is free (project_v done)
                        xt_next = xt_pool.tile([128, HC, NT, 128], bf16, tag="xt")
                        transpose_block(xt_next, x16_next, 0)
                        transpose_block(xt_next, x16_next, 1)
                    if jb == 3 and fetch_next:
                        transpose_block(xt_next, x16_next, 2)
                        transpose_block(xt_next, x16_next, 3)
                        q8n, k8n = qk8_tiles()
                        for j in range(NBLK):
                            qk_block(q8n, k8n, xt_next, j)
                        if early_load and rep + 2 < repeats:
                            # head start for rep i+2's load: the x16 buffer
                            # it overwrites (rep i's) was last read by the
                            # transposes emitted this rep at jb2-3, which
                            # retire well before these DMAs land
                            x16_nn = x16_pool.tile([128, NT, H], bf16, tag="x16")
                            load_block(x16_nn, 0)
                            load_block(x16_nn, 1)
                        else:
                            x16_nn = None
                    if prev is not None:
                        pass2(prev[0], prev[1], outbuf)
                    prev = (jb, at)
                pass2(prev[0], prev[1], outbuf)
                nc.sync.dma_start(
                    out=OUT.ap().rearrange("(t p) d -> p t d", p=128), in_=outbuf
                )
                if fetch_next:
                    x16 = x16_next
                    x16_next = x16_nn if early_load else None
                    xt, q8, k8 = xt_next, q8n, k8n

    nc.compile()
    return nc


_NC = None


def kernel(x, mask, Wq, bq, Wk, bk, Wv, bv):
    global _NC
    if _NC is None:
        _NC = build_nc()
    from concourse.bass_utils import run_bass_kernel_spmd

    x = np.ascontiguousarray(np.asarray(x, dtype=np.float32))
    mask = np.ascontiguousarray(np.asarray(mask, dtype=np.int32))
    shared = {
        "Wq": np.asarray(Wq, np.float32), "bq": np.asarray(bq, np.float32),
        "Wk": np.asarray(Wk, np.float32), "bk": np.asarray(bk, np.float32),
        "Wv": np.asarray(Wv, np.float32), "bv": np.asarray(bv, np.float32),
    }
    in_maps = [dict(x_b=x[c], mask_b=mask[c], **shared) for c in range(B)]
    # the device occasionally wedges transiently (NRT_EXEC_UNIT_UNRECOVERABLE);
    # a retry on a fresh execution recovers it
    last_err = None
    for attempt in range(3):
        try:
            res = run_bass_kernel_spmd(_NC, in_maps, core_ids=list(range(B)))
            return np.stack([res.results[c]["out_b"] for c in range(B)], axis=0)
        except Exception as e:  # noqa: BLE001
            last_err = e
            import time as _time

            _time.sleep(2.0 * (attempt + 1))
    raise last_err


# revision 22
# speedup vs baseline: 1.0221x; 1.0221x over previous
"""Single-head attention (B=8, S=2048, H=1024, D=64) on 8 TRN2 NeuronCores.

Sharding: data-parallel over the batch dim - core b computes batch element b.

v5: fp16 datapath + cross-rep software pipeline.
  Engine budget per rep: PE ~49us of matmul, ScalarE ~33us of exp, Pool
  ~24us of f32->f16 convert, DMA ~24us of x load. The binding constraint
  is PE, but only if the ScalarE-bound attention phase overlaps the
  PE-bound prep phase. v4 ran prep (transpose + qk proj) at the top of
  each rep with ScalarE idle, then attention with the PE half idle. v5
  interleaves rep i+1's prep into rep i's attention blocks:
    jb0: v-projection(i)        jb1: x load+convert(i+1)
    jb2: transpose(i+1) b0-1    jb3: transpose(i+1) b2-3 + qk(i+1) b0-3
  so in steady state every engine streams continuously.

  fp16 instead of bf16: same PE/DVE throughput (1 cyc/row, 2x DVE), but
  all tensor magnitudes here are < 500 so the 3 extra mantissa bits are
  free accuracy (rel err 4.4e-4 vs 3.5e-3).

  fp8_scores=True computes scores with e4m3 q/k in DoubleRow perf mode
  (2 contraction rows/cycle; rel err 1.31e-2, still under the 2e-2
  gate). Measured on HW it is timing-neutral -- the PE does not reach
  the modeled 2x on these 256-cycle matmuls -- so it stays off.

  The prologue (and the repeats=1 graded path) chains per s-block:
  load(b) -> convert(b) -> transpose(b) -> qk(b), so attention starts
  after ~1.5 blocks of load instead of after the full x load.

Dataflow per core (all PE operands f16, f32 PSUM accumulation):
  xT[h, s]  = PE-transpose of f16 x tiles
  q8/k8     = wqk.T @ xT (+bias on PSUM evac, direct to [64, slot, S])
  vT        = wv.T @ xT (+bias); v_aug = [v*mask | mask | 0pad] in [k, 128]
  attnT     = exp(scoresT/8)                        (f16, ScalarE)
  outT_aug  = sum_k v_aug.T @ attnT                 ([128, SB] f32 PSUM)
  out       = transpose(outT_aug) * 1/denominator, one f32 store/rep
"""

import sys

sys.path.insert(0, "/opt/trn_rl_repo")

import numpy as np

B, S, H, D = 8, 2048, 1024, 64
SB = 512          # s-block (streaming block of queries)
NBLK = S // SB    # 4
NT = S // 128     # 16 t-tiles (and s-tiles)
HC = H // 128     # 8 h-chunks


def build_nc(repeats=1, probe_no_load=False, load_engines=("sync",),
             fp8_scores=False, cast_load=False, probe_load_only=False,
             early_load=False, conv_split=False, at_bufs=2):
    import concourse.bacc as bacc
    import concourse.mybir as mybir
    import concourse.tile as tile
    from concourse.masks import make_identity

    dt = mybir.dt
    f32, bf16, i32 = dt.float32, dt.float16, dt.int32
    f8 = dt.float8e4
    DR = mybir.MatmulPerfMode.DoubleRow
    AF = mybir.ActivationFunctionType

    nc = bacc.Bacc("TRN2", target_bir_lowering=False, debug=False, num_devices=8)

    X = nc.dram_tensor("x_b", [S, H], f32, kind="ExternalInput")
    MASK = nc.dram_tensor("mask_b", [S], i32, kind="ExternalInput")
    WQ = nc.dram_tensor("Wq", [H, D], f32, kind="ExternalInput")
    BQ = nc.dram_tensor("bq", [D], f32, kind="ExternalInput")
    WK = nc.dram_tensor("Wk", [H, D], f32, kind="ExternalInput")
    BK = nc.dram_tensor("bk", [D], f32, kind="ExternalInput")
    WV = nc.dram_tensor("Wv", [H, D], f32, kind="ExternalInput")
    BV = nc.dram_tensor("bv", [D], f32, kind="ExternalInput")
    OUT = nc.dram_tensor("out_b", [S, D], f32, kind="ExternalOutput")

    with tile.TileContext(nc) as tc:
        with (
            tc.tile_pool(name="const", bufs=1) as cpool,
            tc.tile_pool(name="xs", bufs=2) as xs_pool,
            tc.tile_pool(name="x16", bufs=2) as x16_pool,
            tc.tile_pool(name="xt", bufs=1) as xt_pool,
            tc.tile_pool(name="stk", bufs=2) as stk_pool,
            tc.tile_pool(name="vps", bufs=2) as v_pool,
            tc.tile_pool(name="attn", bufs=at_bufs) as at_pool,
            tc.tile_pool(name="outs", bufs=2) as o_pool,
            tc.tile_pool(name="ps_tr", bufs=2, space="PSUM") as ps_tr,
            tc.tile_pool(name="ps_sc", bufs=2, space="PSUM") as ps_sc,
            tc.tile_pool(name="ps_acc", bufs=2, space="PSUM") as ps_acc,
        ):
            # ---- constants ----
            ident_f = cpool.tile([128, 128], f32)
            make_identity(nc, ident_f)
            ident = cpool.tile([128, 128], bf16)
            nc.vector.tensor_copy(ident, ident_f)

            # fused [Wq | Wk] stationary (bf16): one projection matmul makes q and k
            wstage = cpool.tile([128, HC, 2 * D], f32, tag="wstage")
            nc.gpsimd.dma_start(out=wstage[:, :, 0:D], in_=WQ.ap().rearrange("(c p) m -> p c m", p=128))
            nc.gpsimd.dma_start(out=wstage[:, :, D:2 * D], in_=WK.ap().rearrange("(c p) m -> p c m", p=128))
            wqk = cpool.tile([128, HC, 2 * D], bf16)
            nc.vector.tensor_copy(wqk, wstage)
            wstage2 = cpool.tile([128, HC, 2 * D], f32, tag="wstage")
            nc.gpsimd.dma_start(out=wstage2[:, :, 0:D], in_=WV.ap().rearrange("(c p) m -> p c m", p=128))
            wv = cpool.tile([128, HC, D], bf16)
            nc.vector.tensor_copy(wv, wstage2[:, :, 0:D])

            bias_qk = cpool.tile([128, 1], f32)
            bias_v = cpool.tile([D, 1], f32)
            nc.gpsimd.dma_start(out=bias_qk[0:D, :], in_=BQ.ap().rearrange("(p o) -> p o", o=1))
            nc.gpsimd.dma_start(out=bias_qk[D:2 * D, :], in_=BK.ap().rearrange("(p o) -> p o", o=1))
            nc.gpsimd.dma_start(out=bias_v, in_=BV.ap().rearrange("(p o) -> p o", o=1))

            # key mask as 1.0/0.0 per t-chunk column (folded into v_aug)
            mask_i = cpool.tile([128, NT], i32)
            nc.gpsimd.dma_start(out=mask_i, in_=MASK.ap().rearrange("(c p) -> p c", p=128))
            mask_f = cpool.tile([128, NT], f32)
            nc.vector.tensor_copy(mask_f, mask_i)
            mask_m = cpool.tile([128, NT], f32)
            nc.vector.tensor_scalar(
                out=mask_m, in0=mask_f,
                scalar1=0.0, scalar2=None,
                op0=mybir.AluOpType.not_equal,
            )

            # v_aug [k-part, t-tile, 128]: cols 0:64 = masked v (rewritten each
            # iteration), col 64 = mask (the softmax-denominator ones column),
            # cols 65:128 = zero pad (full 128 stationary cols enable the PE's
            # fast-weight-load path). Cols 64:128 are written ONCE here.
            v_aug = cpool.tile([128, NT, 128], bf16, tag="v_aug")
            nc.vector.memset(v_aug[:, :, D + 1:128], 0.0)
            for i in range(NT):
                nc.vector.tensor_copy(v_aug[:, i, D:D + 1], mask_m[:, i:i + 1])

            # ---- pipeline stage helpers ----
            def load_block(x16, jb):
                # 2 DMAs (+ Pool converts unless the DMA itself casts).
                # cast_load="split" puts one DMA on the gpsimd SWDGE queue
                # (casting) and one on sync HWDGE (+convert) so the two
                # queues drain the load concurrently.
                for u in range(2):
                    if cast_load is True or (cast_load == "split" and u == 0):
                        # gpsimd SWDGE casts f32->f16 in the DMA CCE: no
                        # staging tile, no Pool convert stage
                        nc.gpsimd.dma_start(
                            out=x16[:, jb * 4 + u * 2:jb * 4 + (u + 1) * 2, :],
                            in_=X.ap().rearrange(
                                "(b u t p) h -> p b u t h", p=128, b=NBLK, u=2
                            )[:, jb, u, :, :],
                        )
                        continue
                    eng = getattr(nc, load_engines[(jb * 2 + u) % len(load_engines)])
                    xs = xs_pool.tile([128, 2, H], f32, tag="xs")
                    eng.dma_start(
                        out=xs,
                        in_=X.ap().rearrange(
                            "(b u t p) h -> p b u t h", p=128, b=NBLK, u=2
                        )[:, jb, u, :, :],
                    )
                    ceng = nc.vector if (conv_split and u == 1) else nc.gpsimd
                    ceng.tensor_copy(
                        x16[:, jb * 4 + u * 2:jb * 4 + (u + 1) * 2, :], xs
                    )

            def transpose_block(xt, x16, jb):
                # PE-transpose the 4 s-tiles x 8 h-chunks of block jb
                for cg in range(HC // 2):
                    ps = ps_tr.tile([128, 2, 4, 128], bf16, tag="tr")
                    for h in range(2):
                        c = 2 * cg + h
                        for st in range(4):
                            nc.tensor.transpose(
                                ps[:, h, st, :],
                                x16[:, jb * 4 + st, c * 128:(c + 1) * 128],
                                ident,
                            )
                    nc.vector.tensor_copy(
                        xt[:, 2 * cg:2 * cg + 2, jb * 4:(jb + 1) * 4, :], ps
                    )

            # q8/k8: fp8-e4m3 copies of qT/kT in DoubleRow layout [64, 2, S].
            # Slot 0 carries data; k8 slot 1 is zeroed so the second DoubleRow
            # contraction subtile contributes nothing (q8 slot 1 is junk -- it
            # multiplies k8's zeros). The zero/junk slots are written only on
            # the first allocation of each physical buffer (bufs=2 cycling).
            qk8_init = {"q8": 0, "k8": 0}

            def qk8_tiles():
                # f16 path needs no DoubleRow slot dim: one slot, no zero
                # fill, half the SBUF
                sdt = f8 if fp8_scores else bf16
                nslot = 2 if fp8_scores else 1
                q8 = stk_pool.tile([D, nslot, S], sdt, tag="q8")
                k8 = stk_pool.tile([D, nslot, S], sdt, tag="k8")
                if fp8_scores and qk8_init["q8"] < 2:
                    qk8_init["q8"] += 1
                    qk8_init["k8"] += 1
                    nc.vector.memset(q8[:, 1, :], 0.0)
                    nc.vector.memset(k8[:, 1, :], 0.0)
                return q8, k8

            def qk_block(q8, k8, xt, j):
                # qk projection; bias-add evacuates PSUM straight to fp8
                # (or f16 when fp8_scores=False)
                sl = slice(j * SB, (j + 1) * SB)
                ps = ps_acc.tile([128, SB], f32, tag="acc")
                for c in range(HC):
                    nc.tensor.matmul(
                        ps, wqk[:, c, :], xt[:, c, j * 4:(j + 1) * 4, :],
                        start=(c == 0), stop=(c == HC - 1),
                    )
                nc.vector.tensor_scalar_add(q8[:, 0, sl], ps[0:D, :], bias_qk[0:D])
                nc.vector.tensor_scalar_add(k8[:, 0, sl], ps[D:2 * D, :], bias_qk[D:2 * D])

            def pass1(jb, q8, k8):
                # fp8 DoubleRow scores (2 contraction rows/cycle) + exp
                sl = slice(jb * SB, (jb + 1) * SB)
                at = at_pool.tile([128, NT // 2, 2, SB], bf16, tag="at")
                for ih in range(NT // 2):
                    ps = ps_sc.tile([128, 2, SB], f32, tag="sc")
                    i0, i1 = ih, ih + NT // 2
                    pm = DR if fp8_scores else None
                    nc.tensor.matmul(
                        ps[:, 0, :],
                        k8[:, 0:(2 if fp8_scores else 1), i0 * 128:(i0 + 1) * 128],
                        q8[:, 0:(2 if fp8_scores else 1), sl],
                        start=True, stop=True, perf_mode=pm,
                    )
                    nc.tensor.matmul(
                        ps[:, 1, :],
                        k8[:, 0:(2 if fp8_scores else 1), i1 * 128:(i1 + 1) * 128],
                        q8[:, 0:(2 if fp8_scores else 1), sl],
                        start=True, stop=True, perf_mode=pm,
                    )
                    nc.scalar.activation(
                        out=at[:, ih, :, :], in_=ps, func=AF.Exp, scale=0.125,
                    )
                return at

            def project_v(xt):
                for j in range(NBLK):
                    sl = slice(j * SB, (j + 1) * SB)
                    ps_v = ps_acc.tile([128, SB], f32, tag="acc")
                    for c in range(HC):
                        nc.tensor.matmul(
                            ps_v[0:D, :], wv[:, c, :], xt[:, c, j * 4:(j + 1) * 4, :],
                            start=(c == 0), stop=(c == HC - 1),
                        )
                    vT = v_pool.tile([D, S], bf16, tag="vT")
                    nc.vector.tensor_scalar_add(vT[:, sl], ps_v[0:D, :], bias_v)
                    pst = ps_tr.tile([128, 2, 4, 128], bf16, tag="tr")
                    for st in range(4):
                        i = j * 4 + st
                        nc.tensor.transpose(
                            pst[:, 0, st, 0:D], vT[:, i * 128:(i + 1) * 128], ident[0:D, 0:D]
                        )
                    for st in range(4):
                        i = j * 4 + st
                        nc.vector.tensor_scalar_mul(
                            v_aug[:, i, 0:D], pst[:, 0, st, 0:D], mask_m[:, i:i + 1]
                        )

            def pass2(jb, at, outbuf):
                # attn@v + transpose + normalize for block jb
                ps_o = ps_acc.tile([128, SB], f32, tag="acc")
                for i in range(NT):
                    nc.tensor.matmul(
                        ps_o, v_aug[:, i, :],
                        at[:, i % (NT // 2), i // (NT // 2), :],
                        start=(i == 0), stop=(i == NT - 1),
                    )
                o_t = o_pool.tile([96, SB], bf16, tag="ot")
                nc.vector.tensor_copy(o_t, ps_o[0:96, :])
                pst = ps_tr.tile([128, 4, 96], bf16, tag="tr")
                for st in range(4):
                    nc.tensor.transpose(
                        pst[:, st, :], o_t[:, st * 128:(st + 1) * 128], ident[0:96, 0:96]
                    )
                recip4 = o_pool.tile([128, 4, 1], f32, tag="recip")
                nc.vector.reciprocal(recip4, pst[:, :, D:D + 1])
                for st in range(4):
                    nc.vector.tensor_scalar_mul(
                        outbuf[:, jb * 4 + st, :], pst[:, st, 0:D], recip4[:, st, :]
                    )

            def make_xt_stack(x16):
                """Transpose + qk-project all blocks, chained per block."""
                xt = xt_pool.tile([128, HC, NT, 128], bf16, tag="xt")
                q8, k8 = qk8_tiles()
                for jb in range(NBLK):
                    transpose_block(xt, x16, jb)
                    qk_block(q8, k8, xt, jb)
                return xt, q8, k8

            if probe_load_only:
                # timing probe: just the x load stream, nothing else
                sink = cpool.tile([128, 1], f32, tag="sink")
                for rep in range(repeats):
                    x16p = x16_pool.tile([128, NT, H], bf16, tag="x16")
                    for jb in range(NBLK):
                        load_block(x16p, jb)
                    nc.vector.tensor_copy(sink, x16p[:, 0, 0:1])
                nc.sync.dma_start(
                    out=OUT.ap().rearrange("(t p) d -> p t d", p=128)[:, 0:1, 0:1],
                    in_=sink.rearrange("p (t o) -> p t o", t=1),
                )
                nc.compile()
                return nc

            # ---- software pipeline across repeats ----
            # prologue: rep 0's load + prep, per-block chained
            x16 = x16_pool.tile([128, NT, H], bf16, tag="x16")
            for jb in range(NBLK):
                load_block(x16, jb)
            xt, q8, k8 = make_xt_stack(x16)

            x16_next = None
            for rep in range(repeats):
                fetch_next = rep + 1 < repeats and not probe_no_load
                outbuf = o_pool.tile([128, NT, D], f32, tag="outbuf")
                prev = None
                for jb in range(NBLK):
                    at = pass1(jb, q8, k8)
                    if jb == 0:
                        project_v(xt)
                        if fetch_next and x16_next is None:
                            # start i+1's x load under the attention phase
                            x16_next = x16_pool.tile([128, NT, H], bf16, tag="x16")
                            load_block(x16_next, 0)
                            load_block(x16_next, 1)
                    if jb == 1 and fetch_next:
                        load_block(x16_next, 2)
                        load_block(x16_next, 3)
                    if jb == 2 and fetch_next:
                        # i+1 prep starts: xt is free (project_v done)
                        xt_next = xt_pool.tile([128, HC, NT, 128], bf16, tag="xt")
                        transpose_block(xt_next, x16_next, 0)
                        transpose_block(xt_next, x16_next, 1)
                    if jb == 3 and fetch_next:
                        transpose_block(xt_next, x16_next, 2)
                        transpose_block(xt_next, x16_next, 3)
                        q8n, k8n = qk8_tiles()
                        for j in range(NBLK):
                            qk_block(q8n, k8n, xt_next, j)
                        if early_load and rep + 2 < repeats:
                            # head start for rep i+2's load: the x16 buffer
                            # it overwrites (rep i's) was last read by the
                            # transposes emitted this rep at jb2-3, which
                            # retire well before these DMAs land
                            x16_nn = x16_pool.tile([128, NT, H], bf16, tag="x16")
                            load_block(x16_nn, 0)
                            load_block(x16_nn, 1)
                        else:
                            x16_nn = None
                    if prev is not None:
                        pass2(prev[0], prev[1], outbuf)
                    prev = (jb, at)
                pass2(prev[0], prev[1], outbuf)
                nc.sync.dma_start(
                    out=OUT.ap().rearrange("(t p) d -> p t d", p=128), in_=outbuf
                )
                if fetch_next:
                    x16 = x16_next
                    x16_next = x16_nn if early_load else None
                    xt, q8, k8 = xt_next, q8n, k8n

    nc.compile()
    return nc


_NC = None


def kernel(x, mask, Wq, bq, Wk, bk, Wv, bv):
    global _NC
    if _NC is None:
        _NC = build_nc()
    from concourse.bass_utils import run_bass_kernel_spmd

    x = np.ascontiguousarray(np.asarray(x, dtype=np.float32))
    mask = np.ascontiguousarray(np.asarray(mask, dtype=np.int32))
    shared = {
        "Wq": np.asarray(Wq, np.float32), "bq": np.asarray(bq, np.float32),
        "Wk": np.asarray(Wk, np.float32), "bk": np.asarray(bk, np.float32),
        "Wv": np.asarray(Wv, np.float32), "bv": np.asarray(bv, np.float32),
    }
    in_maps = [dict(x_b=x[c], mask_b=mask[c], **shared) for c in range(B)]
    # the device occasionally wedges transiently (NRT_EXEC_UNIT_UNRECOVERABLE);
    # a retry on a fresh execution recovers it
    last_err = None
    for attempt in range(3):
        try:
            res = run_bass_kernel_spmd(_NC, in_maps, core_ids=list(range(B)))
            return np.stack([res.results[c]["out_b"] for c in range(B)], axis=0)
        except Exception as e:  # noqa: BLE001
            last_err = e
            import time as _time

            _time.sleep(2.0 * (attempt + 1))
    raise last_err


# revision 24
# speedup vs baseline: 1.0237x; 1.0016x over previous
"""Single-head attention (B=8, S=2048, H=1024, D=64) on 8 TRN2 NeuronCores.

Sharding: data-parallel over the batch dim - core b computes batch element b.

v5: fp16 datapath + cross-rep software pipeline.
  Engine budget per rep: PE ~49us of matmul, ScalarE ~33us of exp, Pool
  ~24us of f32->f16 convert, DMA ~24us of x load. The binding constraint
  is PE, but only if the ScalarE-bound attention phase overlaps the
  PE-bound prep phase. v4 ran prep (transpose + qk proj) at the top of
  each rep with ScalarE idle, then attention with the PE half idle. v5
  interleaves rep i+1's prep into rep i's attention blocks:
    jb0: v-projection(i)        jb1: x load+convert(i+1)
    jb2: transpose(i+1) b0-1    jb3: transpose(i+1) b2-3 + qk(i+1) b0-3
  so in steady state every engine streams continuously.

  fp16 instead of bf16: same PE/DVE throughput (1 cyc/row, 2x DVE), but
  all tensor magnitudes here are < 500 so the 3 extra mantissa bits are
  free accuracy (rel err 4.4e-4 vs 3.5e-3).

  fp8_scores=True computes scores with e4m3 q/k in DoubleRow perf mode
  (2 contraction rows/cycle; rel err 1.31e-2, still under the 2e-2
  gate). Measured on HW it is timing-neutral -- the PE does not reach
  the modeled 2x on these 256-cycle matmuls -- so it stays off.

  The prologue (and the repeats=1 graded path) chains per s-block:
  load(b) -> convert(b) -> transpose(b) -> qk(b), so attention starts
  after ~1.5 blocks of load instead of after the full x load.

Dataflow per core (all PE operands f16, f32 PSUM accumulation):
  xT[h, s]  = PE-transpose of f16 x tiles
  q8/k8     = wqk.T @ xT (+bias on PSUM evac, direct to [64, slot, S])
  vT        = wv.T @ xT (+bias); v_aug = [v*mask | mask | 0pad] in [k, 128]
  attnT     = exp(scoresT/8)                        (f16, ScalarE)
  outT_aug  = sum_k v_aug.T @ attnT                 ([128, SB] f32 PSUM)
  out       = transpose(outT_aug) * 1/denominator, one f32 store/rep
"""

import sys

sys.path.insert(0, "/opt/trn_rl_repo")

import numpy as np

B, S, H, D = 8, 2048, 1024, 64
SB = 512          # s-block (streaming block of queries)
NBLK = S // SB    # 4
NT = S // 128     # 16 t-tiles (and s-tiles)
HC = H // 128     # 8 h-chunks


def build_nc(repeats=1, probe_no_load=False, load_engines=("sync",),
             fp8_scores=False, cast_load=False, probe_load_only=False,
             early_load=False, conv_split=False, at_bufs=3, xs_bufs=2):
    import concourse.bacc as bacc
    import concourse.mybir as mybir
    import concourse.tile as tile
    from concourse.masks import make_identity

    dt = mybir.dt
    f32, bf16, i32 = dt.float32, dt.float16, dt.int32
    f8 = dt.float8e4
    DR = mybir.MatmulPerfMode.DoubleRow
    AF = mybir.ActivationFunctionType

    nc = bacc.Bacc("TRN2", target_bir_lowering=False, debug=False, num_devices=8)

    X = nc.dram_tensor("x_b", [S, H], f32, kind="ExternalInput")
    MASK = nc.dram_tensor("mask_b", [S], i32, kind="ExternalInput")
    WQ = nc.dram_tensor("Wq", [H, D], f32, kind="ExternalInput")
    BQ = nc.dram_tensor("bq", [D], f32, kind="ExternalInput")
    WK = nc.dram_tensor("Wk", [H, D], f32, kind="ExternalInput")
    BK = nc.dram_tensor("bk", [D], f32, kind="ExternalInput")
    WV = nc.dram_tensor("Wv", [H, D], f32, kind="ExternalInput")
    BV = nc.dram_tensor("bv", [D], f32, kind="ExternalInput")
    OUT = nc.dram_tensor("out_b", [S, D], f32, kind="ExternalOutput")

    with tile.TileContext(nc) as tc:
        with (
            tc.tile_pool(name="const", bufs=1) as cpool,
            tc.tile_pool(name="xs", bufs=xs_bufs) as xs_pool,
            tc.tile_pool(name="x16", bufs=2) as x16_pool,
            tc.tile_pool(name="xt", bufs=1) as xt_pool,
            tc.tile_pool(name="stk", bufs=2) as stk_pool,
            tc.tile_pool(name="vps", bufs=2) as v_pool,
            tc.tile_pool(name="attn", bufs=at_bufs) as at_pool,
            tc.tile_pool(name="outs", bufs=2) as o_pool,
            tc.tile_pool(name="ps_tr", bufs=2, space="PSUM") as ps_tr,
            tc.tile_pool(name="ps_sc", bufs=2, space="PSUM") as ps_sc,
            tc.tile_pool(name="ps_acc", bufs=2, space="PSUM") as ps_acc,
        ):
            # ---- constants ----
            ident_f = cpool.tile([128, 128], f32)
            make_identity(nc, ident_f)
            ident = cpool.tile([128, 128], bf16)
            nc.vector.tensor_copy(ident, ident_f)

            # fused [Wq | Wk] stationary (bf16): one projection matmul makes q and k
            wstage = cpool.tile([128, HC, 2 * D], f32, tag="wstage")
            nc.gpsimd.dma_start(out=wstage[:, :, 0:D], in_=WQ.ap().rearrange("(c p) m -> p c m", p=128))
            nc.gpsimd.dma_start(out=wstage[:, :, D:2 * D], in_=WK.ap().rearrange("(c p) m -> p c m", p=128))
            wqk = cpool.tile([128, HC, 2 * D], bf16)
            nc.vector.tensor_copy(wqk, wstage)
            wstage2 = cpool.tile([128, HC, 2 * D], f32, tag="wstage")
            nc.gpsimd.dma_start(out=wstage2[:, :, 0:D], in_=WV.ap().rearrange("(c p) m -> p c m", p=128))
            wv = cpool.tile([128, HC, D], bf16)
            nc.vector.tensor_copy(wv, wstage2[:, :, 0:D])

            bias_qk = cpool.tile([128, 1], f32)
            bias_v = cpool.tile([D, 1], f32)
            nc.gpsimd.dma_start(out=bias_qk[0:D, :], in_=BQ.ap().rearrange("(p o) -> p o", o=1))
            nc.gpsimd.dma_start(out=bias_qk[D:2 * D, :], in_=BK.ap().rearrange("(p o) -> p o", o=1))
            nc.gpsimd.dma_start(out=bias_v, in_=BV.ap().rearrange("(p o) -> p o", o=1))

            # key mask as 1.0/0.0 per t-chunk column (folded into v_aug)
            mask_i = cpool.tile([128, NT], i32)
            nc.gpsimd.dma_start(out=mask_i, in_=MASK.ap().rearrange("(c p) -> p c", p=128))
            mask_f = cpool.tile([128, NT], f32)
            nc.vector.tensor_copy(mask_f, mask_i)
            mask_m = cpool.tile([128, NT], f32)
            nc.vector.tensor_scalar(
                out=mask_m, in0=mask_f,
                scalar1=0.0, scalar2=None,
                op0=mybir.AluOpType.not_equal,
            )

            # v_aug [k-part, t-tile, 128]: cols 0:64 = masked v (rewritten each
            # iteration), col 64 = mask (the softmax-denominator ones column),
            # cols 65:128 = zero pad (full 128 stationary cols enable the PE's
            # fast-weight-load path). Cols 64:128 are written ONCE here.
            v_aug = cpool.tile([128, NT, 128], bf16, tag="v_aug")
            nc.vector.memset(v_aug[:, :, D + 1:128], 0.0)
            for i in range(NT):
                nc.vector.tensor_copy(v_aug[:, i, D:D + 1], mask_m[:, i:i + 1])

            # ---- pipeline stage helpers ----
            def load_block(x16, jb):
                # 2 DMAs (+ Pool converts unless the DMA itself casts).
                # cast_load="split" puts one DMA on the gpsimd SWDGE queue
                # (casting) and one on sync HWDGE (+convert) so the two
                # queues drain the load concurrently.
                for u in range(2):
                    if cast_load is True or (cast_load == "split" and u == 0):
                        # gpsimd SWDGE casts f32->f16 in the DMA CCE: no
                        # staging tile, no Pool convert stage
                        nc.gpsimd.dma_start(
                            out=x16[:, jb * 4 + u * 2:jb * 4 + (u + 1) * 2, :],
                            in_=X.ap().rearrange(
                                "(b u t p) h -> p b u t h", p=128, b=NBLK, u=2
                            )[:, jb, u, :, :],
                        )
                        continue
                    eng = getattr(nc, load_engines[(jb * 2 + u) % len(load_engines)])
                    xs = xs_pool.tile([128, 2, H], f32, tag="xs")
                    eng.dma_start(
                        out=xs,
                        in_=X.ap().rearrange(
                            "(b u t p) h -> p b u t h", p=128, b=NBLK, u=2
                        )[:, jb, u, :, :],
                    )
                    ceng = nc.vector if (conv_split and u == 1) else nc.gpsimd
                    ceng.tensor_copy(
                        x16[:, jb * 4 + u * 2:jb * 4 + (u + 1) * 2, :], xs
                    )

            def transpose_block(xt, x16, jb):
                # PE-transpose the 4 s-tiles x 8 h-chunks of block jb
                for cg in range(HC // 2):
                    ps = ps_tr.tile([128, 2, 4, 128], bf16, tag="tr")
                    for h in range(2):
                        c = 2 * cg + h
                        for st in range(4):
                            nc.tensor.transpose(
                                ps[:, h, st, :],
                                x16[:, jb * 4 + st, c * 128:(c + 1) * 128],
                                ident,
                            )
                    nc.vector.tensor_copy(
                        xt[:, 2 * cg:2 * cg + 2, jb * 4:(jb + 1) * 4, :], ps
                    )

            # q8/k8: fp8-e4m3 copies of qT/kT in DoubleRow layout [64, 2, S].
            # Slot 0 carries data; k8 slot 1 is zeroed so the second DoubleRow
            # contraction subtile contributes nothing (q8 slot 1 is junk -- it
            # multiplies k8's zeros). The zero/junk slots are written only on
            # the first allocation of each physical buffer (bufs=2 cycling).
            qk8_init = {"q8": 0, "k8": 0}

            def qk8_tiles():
                # f16 path needs no DoubleRow slot dim: one slot, no zero
                # fill, half the SBUF
                sdt = f8 if fp8_scores else bf16
                nslot = 2 if fp8_scores else 1
                q8 = stk_pool.tile([D, nslot, S], sdt, tag="q8")
                k8 = stk_pool.tile([D, nslot, S], sdt, tag="k8")
                if fp8_scores and qk8_init["q8"] < 2:
                    qk8_init["q8"] += 1
                    qk8_init["k8"] += 1
                    nc.vector.memset(q8[:, 1, :], 0.0)
                    nc.vector.memset(k8[:, 1, :], 0.0)
                return q8, k8

            def qk_block(q8, k8, xt, j):
                # qk projection; bias-add evacuates PSUM straight to fp8
                # (or f16 when fp8_scores=False)
                sl = slice(j * SB, (j + 1) * SB)
                ps = ps_acc.tile([128, SB], f32, tag="acc")
                for c in range(HC):
                    nc.tensor.matmul(
                        ps, wqk[:, c, :], xt[:, c, j * 4:(j + 1) * 4, :],
                        start=(c == 0), stop=(c == HC - 1),
                    )
                nc.vector.tensor_scalar_add(q8[:, 0, sl], ps[0:D, :], bias_qk[0:D])
                nc.vector.tensor_scalar_add(k8[:, 0, sl], ps[D:2 * D, :], bias_qk[D:2 * D])

            def pass1(jb, q8, k8):
                # fp8 DoubleRow scores (2 contraction rows/cycle) + exp
                sl = slice(jb * SB, (jb + 1) * SB)
                at = at_pool.tile([128, NT // 2, 2, SB], bf16, tag="at")
                for ih in range(NT // 2):
                    ps = ps_sc.tile([128, 2, SB], f32, tag="sc")
                    i0, i1 = ih, ih + NT // 2
                    pm = DR if fp8_scores else None
                    nc.tensor.matmul(
                        ps[:, 0, :],
                        k8[:, 0:(2 if fp8_scores else 1), i0 * 128:(i0 + 1) * 128],
                        q8[:, 0:(2 if fp8_scores else 1), sl],
                        start=True, stop=True, perf_mode=pm,
                    )
                    nc.tensor.matmul(
                        ps[:, 1, :],
                        k8[:, 0:(2 if fp8_scores else 1), i1 * 128:(i1 + 1) * 128],
                        q8[:, 0:(2 if fp8_scores else 1), sl],
                        start=True, stop=True, perf_mode=pm,
                    )
                    nc.scalar.activation(
                        out=at[:, ih, :, :], in_=ps, func=AF.Exp, scale=0.125,
                    )
                return at

            def project_v(xt):
                for j in range(NBLK):
                    sl = slice(j * SB, (j + 1) * SB)
                    ps_v = ps_acc.tile([128, SB], f32, tag="acc")
                    for c in range(HC):
                        nc.tensor.matmul(
                            ps_v[0:D, :], wv[:, c, :], xt[:, c, j * 4:(j + 1) * 4, :],
                            start=(c == 0), stop=(c == HC - 1),
                        )
                    vT = v_pool.tile([D, S], bf16, tag="vT")
                    nc.vector.tensor_scalar_add(vT[:, sl], ps_v[0:D, :], bias_v)
                    pst = ps_tr.tile([128, 2, 4, 128], bf16, tag="tr")
                    for st in range(4):
                        i = j * 4 + st
                        nc.tensor.transpose(
                            pst[:, 0, st, 0:D], vT[:, i * 128:(i + 1) * 128], ident[0:D, 0:D]
                        )
                    for st in range(4):
                        i = j * 4 + st
                        nc.vector.tensor_scalar_mul(
                            v_aug[:, i, 0:D], pst[:, 0, st, 0:D], mask_m[:, i:i + 1]
                        )

            def pass2(jb, at, outbuf):
                # attn@v + transpose + normalize for block jb
                ps_o = ps_acc.tile([128, SB], f32, tag="acc")
                for i in range(NT):
                    nc.tensor.matmul(
                        ps_o, v_aug[:, i, :],
                        at[:, i % (NT // 2), i // (NT // 2), :],
                        start=(i == 0), stop=(i == NT - 1),
                    )
                o_t = o_pool.tile([96, SB], bf16, tag="ot")
                nc.vector.tensor_copy(o_t, ps_o[0:96, :])
                pst = ps_tr.tile([128, 4, 96], bf16, tag="tr")
                for st in range(4):
                    nc.tensor.transpose(
                        pst[:, st, :], o_t[:, st * 128:(st + 1) * 128], ident[0:96, 0:96]
                    )
                recip4 = o_pool.tile([128, 4, 1], f32, tag="recip")
                nc.vector.reciprocal(recip4, pst[:, :, D:D + 1])
                for st in range(4):
                    nc.vector.tensor_scalar_mul(
                        outbuf[:, jb * 4 + st, :], pst[:, st, 0:D], recip4[:, st, :]
                    )

            def make_xt_stack(x16):
                """Transpose + qk-project all blocks, chained per block."""
                xt = xt_pool.tile([128, HC, NT, 128], bf16, tag="xt")
                q8, k8 = qk8_tiles()
                for jb in range(NBLK):
                    transpose_block(xt, x16, jb)
                    qk_block(q8, k8, xt, jb)
                return xt, q8, k8

            if probe_load_only:
                # timing probe: just the x load stream, nothing else
                sink = cpool.tile([128, 1], f32, tag="sink")
                for rep in range(repeats):
                    x16p = x16_pool.tile([128, NT, H], bf16, tag="x16")
                    for jb in range(NBLK):
                        load_block(x16p, jb)
                    nc.vector.tensor_copy(sink, x16p[:, 0, 0:1])
                nc.sync.dma_start(
                    out=OUT.ap().rearrange("(t p) d -> p t d", p=128)[:, 0:1, 0:1],
                    in_=sink.rearrange("p (t o) -> p t o", t=1),
                )
                nc.compile()
                return nc

            # ---- software pipeline across repeats ----
            # prologue: rep 0's load + prep, per-block chained
            x16 = x16_pool.tile([128, NT, H], bf16, tag="x16")
            for jb in range(NBLK):
                load_block(x16, jb)
            xt, q8, k8 = make_xt_stack(x16)

            x16_next = None
            for rep in range(repeats):
                fetch_next = rep + 1 < repeats and not probe_no_load
                outbuf = o_pool.tile([128, NT, D], f32, tag="outbuf")
                prev = None
                for jb in range(NBLK):
                    at = pass1(jb, q8, k8)
                    if jb == 0:
                        project_v(xt)
                        if fetch_next and x16_next is None:
                            # start i+1's x load under the attention phase
                            x16_next = x16_pool.tile([128, NT, H], bf16, tag="x16")
                            load_block(x16_next, 0)
                            load_block(x16_next, 1)
                    if jb == 1 and fetch_next:
                        load_block(x16_next, 2)
                        load_block(x16_next, 3)
                    if jb == 2 and fetch_next:
                        # i+1 prep starts: xt is free (project_v done)
                        xt_next = xt_pool.tile([128, HC, NT, 128], bf16, tag="xt")
                        transpose_block(xt_next, x16_next, 0)
                        transpose_block(xt_next, x16_next, 1)
                    if jb == 3 and fetch_next:
                        transpose_block(xt_next, x16_next, 2)
                        transpose_block(xt_next, x16_next, 3)
                        q8n, k8n = qk8_tiles()
                        for j in range(NBLK):
                            qk_block(q8n, k8n, xt_next, j)
                        if early_load and rep + 2 < repeats:
                            # head start for rep i+2's load: the x16 buffer
                            # it overwrites (rep i's) was last read by the
                            # transposes emitted this rep at jb2-3, which
                            # retire well before these DMAs land
                            x16_nn = x16_pool.tile([128, NT, H], bf16, tag="x16")
                            load_block(x16_nn, 0)
                            load_block(x16_nn, 1)
                        else:
                            x16_nn = None
                    if prev is not None:
                        pass2(prev[0], prev[1], outbuf)
                    prev = (jb, at)
                pass2(prev[0], prev[1], outbuf)
                nc.sync.dma_start(
                    out=OUT.ap().rearrange("(t p) d -> p t d", p=128), in_=outbuf
                )
                if fetch_next:
                    x16 = x16_next
                    x16_next = x16_nn if early_load else None
                    xt, q8, k8 = xt_next, q8n, k8n

    nc.compile()
    return nc


_NC = None


def kernel(x, mask, Wq, bq, Wk, bk, Wv, bv):
    global _NC
    if _NC is None:
        _NC = build_nc()
    from concourse.bass_utils import run_bass_kernel_spmd

    x = np.ascontiguousarray(np.asarray(x, dtype=np.float32))
    mask = np.ascontiguousarray(np.asarray(mask, dtype=np.int32))
    shared = {
        "Wq": np.asarray(Wq, np.float32), "bq": np.asarray(bq, np.float32),
        "Wk": np.asarray(Wk, np.float32), "bk": np.asarray(bk, np.float32),
        "Wv": np.asarray(Wv, np.float32), "bv": np.asarray(bv, np.float32),
    }
    in_maps = [dict(x_b=x[c], mask_b=mask[c], **shared) for c in range(B)]
    # the device occasionally wedges transiently (NRT_EXEC_UNIT_UNRECOVERABLE);
    # a retry on a fresh execution recovers it
    last_err = None
    for attempt in range(3):
        try:
            res = run_bass_kernel_spmd(_NC, in_maps, core_ids=list(range(B)))
            return np.stack([res.results[c]["out_b"] for c in range(B)], axis=0)
        except Exception as e:  # noqa: BLE001
            last_err = e
            import time as _time

            _time.sleep(2.0 * (attempt + 1))
    raise last_err
